# revision 7
# baseline (speedup 1.0000x reference)
"""Trainium2 Bass kernel for nn_MessageFunction (GNN message passing).

reference:
    edge_out = einsum('ben,em->bmn', e_vw, W_e) + b_e   # [B, 128, N]
    node_out = einsum('bfn,fm->bmn', h_w,  W_n) + b_n   # [B, 128, N]
    out      = relu(concat([edge_out, node_out], axis=1))  # [B, 256, N]

h_v is an unused input (dead in the reference) — never transferred.

Sharding: data-parallel over the node axis (last dim) across 8 cores,
weights/biases replicated. Each core handles 6250 nodes:
  - stream [128, 1250] fp32 tiles of e_vw / h_w per batch,
  - matmul against resident 128x128 weights (fp32, K=128 contraction),
  - bias + ReLU fused: edge half on ScalarE (activation Relu w/ bias),
    node half on VectorE (tensor_scalar add+max) so the two engines
    run in parallel,
  - one combined [128, 2, 1250] DMA writes both halves of the output.
"""

import numpy as np

import concourse.bass as bass
import concourse.mybir as mybir
import concourse.tile as tile
from concourse import bacc
from concourse.bass_utils import run_bass_kernel_spmd

N_CORES = 8
B = 4
F = 128      # EDGE_F == NODE_F (contraction dim)
HALF = 128   # output channels per linear
N_NODES = 50000
NS = N_NODES // N_CORES       # 6250 nodes per core
T_DMA = 1250                  # free-dim tile per DMA (5 per batch slab)
N_T = NS // T_DMA             # 5
MM_SPLITS = (417, 417, 416)   # PSUM bank limit: <=512 fp32 per matmul

_FP32 = mybir.dt.float32

_compiled = None


def _build():
    nc = bacc.Bacc(
        "TRN2",
        target_bir_lowering=False,
        debug=False,
        num_devices=N_CORES,
    )
    e_vw = nc.dram_tensor("e_vw", (B, F, NS), _FP32, kind="ExternalInput").ap()
    h_w = nc.dram_tensor("h_w", (B, F, NS), _FP32, kind="ExternalInput").ap()
    W_e = nc.dram_tensor("W_e", (F, HALF), _FP32, kind="ExternalInput").ap()
    W_n = nc.dram_tensor("W_n", (F, HALF), _FP32, kind="ExternalInput").ap()
    b_e = nc.dram_tensor("b_e", (HALF, 1), _FP32, kind="ExternalInput").ap()
    b_n = nc.dram_tensor("b_n", (HALF, 1), _FP32, kind="ExternalInput").ap()
    out = nc.dram_tensor("out", (B, 2 * HALF, NS), _FP32, kind="ExternalOutput").ap()

    relu = mybir.ActivationFunctionType.Relu
    alu_add = mybir.AluOpType.add
    alu_max = mybir.AluOpType.max

    with tile.TileContext(nc) as tc:
        with (
            tc.tile_pool(name="consts", bufs=1) as cpool,
            tc.tile_pool(name="xin", bufs=8) as inpool,
            tc.tile_pool(name="xout", bufs=4) as outpool,
            tc.tile_pool(name="psum", bufs=8, space="PSUM") as pspool,
        ):
            w_e_sb = cpool.tile([F, HALF], _FP32, tag="w_e")
            w_n_sb = cpool.tile([F, HALF], _FP32, tag="w_n")
            b_e_sb = cpool.tile([HALF, 1], _FP32, tag="b_e")
            b_n_sb = cpool.tile([HALF, 1], _FP32, tag="b_n")
            # constants via SWDGE so the sync HWDGE ring starts with the
            # first big streaming load
            nc.gpsimd.dma_start(w_e_sb[:], W_e)
            nc.gpsimd.dma_start(w_n_sb[:], W_n)
            nc.gpsimd.dma_start(b_e_sb[:], b_e)
            nc.gpsimd.dma_start(b_n_sb[:], b_n)

            for bb in range(B):
                for t in range(N_T):
                    sl = bass.ts(t, T_DMA)
                    e_t = inpool.tile([F, T_DMA], _FP32, tag="e")
                    h_t = inpool.tile([F, T_DMA], _FP32, tag="h")
                    nc.sync.dma_start(e_t[:], e_vw[bb, :, sl])
                    nc.sync.dma_start(h_t[:], h_w[bb, :, sl])
                    o_e = outpool.tile([F, T_DMA], _FP32, tag="oe")
                    o_n = outpool.tile([F, T_DMA], _FP32, tag="on")
                    # all edge matmuls first, then all node matmuls: fewer
                    # weight-buffer alternations on PE
                    c0 = 0
                    for w in MM_SPLITS:
                        ps_e = pspool.tile([HALF, 512], _FP32, tag="ps")
                        nc.tensor.matmul(ps_e[:, :w], w_e_sb[:], e_t[:, c0 : c0 + w])
                        nc.scalar.activation(
                            o_e[:, c0 : c0 + w],
                            ps_e[:, :w],
                            relu,
                            bias=b_e_sb[:, 0:1],
                        )
                        c0 += w
                    # edge-half store from ACT's HWDGE ring: depends only on
                    # ACT's own output, so no cross-engine HOL
                    nc.scalar.dma_start(out[bb, 0:HALF, sl], o_e[:])
                    c0 = 0
                    for w in MM_SPLITS:
                        ps_n = pspool.tile([HALF, 512], _FP32, tag="ps")
                        nc.tensor.matmul(ps_n[:, :w], w_n_sb[:], h_t[:, c0 : c0 + w])
                        nc.vector.tensor_scalar(
                            o_n[:, c0 : c0 + w],
                            ps_n[:, :w],
                            b_n_sb[:, 0:1],
                            0.0,
                            alu_add,
                            alu_max,
                        )
                        c0 += w
                    # node-half store on SWDGE (gpsimd is otherwise idle)
                    nc.gpsimd.dma_start(out[bb, HALF : 2 * HALF, sl], o_n[:])

    nc.compile()
    return nc


def _get_nc():
    global _compiled
    if _compiled is None:
        _compiled = _build()
    return _compiled


def run(h_w, e_vw, W_e, b_e, W_n, b_n, trace=False, **kwargs):
    nc = _get_nc()
    h_w = np.ascontiguousarray(np.asarray(h_w, dtype=np.float32))
    e_vw = np.ascontiguousarray(np.asarray(e_vw, dtype=np.float32))
    w_e = np.ascontiguousarray(np.asarray(W_e, dtype=np.float32))
    w_n = np.ascontiguousarray(np.asarray(W_n, dtype=np.float32))
    be = np.ascontiguousarray(np.asarray(b_e, dtype=np.float32).reshape(HALF, 1))
    bn = np.ascontiguousarray(np.asarray(b_n, dtype=np.float32).reshape(HALF, 1))

    in_maps = []
    for c in range(N_CORES):
        sl = slice(c * NS, (c + 1) * NS)
        in_maps.append(
            {
                "e_vw": np.ascontiguousarray(e_vw[:, :, sl]),
                "h_w": np.ascontiguousarray(h_w[:, :, sl]),
                "W_e": w_e,
                "W_n": w_n,
                "b_e": be,
                "b_n": bn,
            }
        )
    res = run_bass_kernel_spmd(
        nc, in_maps, core_ids=list(range(N_CORES)), trace=trace, **kwargs
    )
    full = np.concatenate([res.results[c]["out"] for c in range(N_CORES)], axis=2)
    return full, res


def kernel(h_v=None, h_w=None, e_vw=None, W_e=None, b_e=None, W_n=None, b_n=None):
    full, _ = run(h_w, e_vw, W_e, b_e, W_n, b_n, trace=False)
    return full


# revision 9
# speedup vs baseline: 1.0469x; 1.0469x over previous
"""Trainium2 Bass kernel for nn_MessageFunction (GNN message passing).

reference:
    edge_out = einsum('ben,em->bmn', e_vw, W_e) + b_e   # [B, 128, N]
    node_out = einsum('bfn,fm->bmn', h_w,  W_n) + b_n   # [B, 128, N]
    out      = relu(concat([edge_out, node_out], axis=1))  # [B, 256, N]

h_v is an unused input (dead in the reference) — never transferred.

Sharding: data-parallel over the node axis (last dim) across 8 cores,
weights/biases replicated. Each core handles 6250 nodes:
  - stream [128, 1250] fp32 tiles of e_vw / h_w per batch,
  - matmul against resident 128x128 weights (fp32, K=128 contraction),
  - bias + ReLU fused: edge half on ScalarE (activation Relu w/ bias),
    node half on VectorE (tensor_scalar add+max) so the two engines
    run in parallel,
  - one combined [128, 2, 1250] DMA writes both halves of the output.
"""

import numpy as np

import concourse.bass as bass
import concourse.mybir as mybir
import concourse.tile as tile
from concourse import bacc
from concourse.bass_utils import run_bass_kernel_spmd

N_CORES = 8
B = 4
F = 128      # EDGE_F == NODE_F (contraction dim)
HALF = 128   # output channels per linear
N_NODES = 50000
NS = N_NODES // N_CORES       # 6250 nodes per core
T_DMA = 1250                  # free-dim tile per DMA (5 per batch slab)
N_T = NS // T_DMA             # 5

# Per-batch tile widths. First/last batch tapered: small tiles at the
# global pipeline start (stores ramp up sooner) and end (shorter drain).
_BODY = [T_DMA] * N_T                     # [1250]*5
_TILES = {
    0: [625, 625] + [T_DMA] * (N_T - 1),  # tapered start
    B - 1: [T_DMA] * (N_T - 1) + [625, 625],  # tapered end
}


def _mm_splits(width):
    # <=512 fp32 per matmul (one PSUM bank); near-uniform splits
    n = -(-width // 512)
    base, rem = divmod(width, n)
    return [base + (1 if i < rem else 0) for i in range(n)]

_FP32 = mybir.dt.float32

_compiled = None


def _build():
    nc = bacc.Bacc(
        "TRN2",
        target_bir_lowering=False,
        debug=False,
        num_devices=N_CORES,
    )
    e_vw = nc.dram_tensor("e_vw", (B, F, NS), _FP32, kind="ExternalInput").ap()
    h_w = nc.dram_tensor("h_w", (B, F, NS), _FP32, kind="ExternalInput").ap()
    W_e = nc.dram_tensor("W_e", (F, HALF), _FP32, kind="ExternalInput").ap()
    W_n = nc.dram_tensor("W_n", (F, HALF), _FP32, kind="ExternalInput").ap()
    b_e = nc.dram_tensor("b_e", (HALF, 1), _FP32, kind="ExternalInput").ap()
    b_n = nc.dram_tensor("b_n", (HALF, 1), _FP32, kind="ExternalInput").ap()
    out = nc.dram_tensor("out", (B, 2 * HALF, NS), _FP32, kind="ExternalOutput").ap()

    relu = mybir.ActivationFunctionType.Relu
    alu_add = mybir.AluOpType.add
    alu_max = mybir.AluOpType.max

    with tile.TileContext(nc) as tc:
        with (
            tc.tile_pool(name="consts", bufs=1) as cpool,
            tc.tile_pool(name="xin", bufs=8) as inpool,
            tc.tile_pool(name="xout", bufs=4) as outpool,
            tc.tile_pool(name="psum", bufs=8, space="PSUM") as pspool,
        ):
            w_e_sb = cpool.tile([F, HALF], _FP32, tag="w_e")
            w_n_sb = cpool.tile([F, HALF], _FP32, tag="w_n")
            b_e_sb = cpool.tile([HALF, 1], _FP32, tag="b_e")
            b_n_sb = cpool.tile([HALF, 1], _FP32, tag="b_n")
            # constants via SWDGE so the sync HWDGE ring starts with the
            # first big streaming load
            nc.gpsimd.dma_start(w_e_sb[:], W_e)
            nc.gpsimd.dma_start(w_n_sb[:], W_n)
            nc.gpsimd.dma_start(b_e_sb[:], b_e)
            nc.gpsimd.dma_start(b_n_sb[:], b_n)

            for bb in range(B):
                n0 = 0
                for width in _TILES.get(bb, _BODY):
                    sl = bass.ds(n0, width)
                    n0 += width
                    e_t = inpool.tile([F, T_DMA], _FP32, tag="e")
                    h_t = inpool.tile([F, T_DMA], _FP32, tag="h")
                    nc.sync.dma_start(e_t[:, :width], e_vw[bb, :, sl])
                    nc.sync.dma_start(h_t[:, :width], h_w[bb, :, sl])
                    o_e = outpool.tile([F, T_DMA], _FP32, tag="oe")
                    o_n = outpool.tile([F, T_DMA], _FP32, tag="on")
                    # all edge matmuls first, then all node matmuls: fewer
                    # weight-buffer alternations on PE
                    c0 = 0
                    for w in _mm_splits(width):
                        ps_e = pspool.tile([HALF, 512], _FP32, tag="ps")
                        nc.tensor.matmul(ps_e[:, :w], w_e_sb[:], e_t[:, c0 : c0 + w])
                        nc.scalar.activation(
                            o_e[:, c0 : c0 + w],
                            ps_e[:, :w],
                            relu,
                            bias=b_e_sb[:, 0:1],
                        )
                        c0 += w
                    # edge-half store from ACT's HWDGE ring: depends only on
                    # ACT's own output, so no cross-engine HOL
                    nc.scalar.dma_start(out[bb, 0:HALF, sl], o_e[:, :width])
                    c0 = 0
                    for w in _mm_splits(width):
                        ps_n = pspool.tile([HALF, 512], _FP32, tag="ps")
                        nc.tensor.matmul(ps_n[:, :w], w_n_sb[:], h_t[:, c0 : c0 + w])
                        nc.vector.tensor_scalar(
                            o_n[:, c0 : c0 + w],
                            ps_n[:, :w],
                            b_n_sb[:, 0:1],
                            0.0,
                            alu_add,
                            alu_max,
                        )
                        c0 += w
                    # node-half store on SWDGE (gpsimd is otherwise idle)
                    nc.gpsimd.dma_start(out[bb, HALF : 2 * HALF, sl], o_n[:, :width])

    nc.compile()
    return nc


def _get_nc():
    global _compiled
    if _compiled is None:
        _compiled = _build()
    return _compiled


def run(h_w, e_vw, W_e, b_e, W_n, b_n, trace=False, **kwargs):
    nc = _get_nc()
    h_w = np.ascontiguousarray(np.asarray(h_w, dtype=np.float32))
    e_vw = np.ascontiguousarray(np.asarray(e_vw, dtype=np.float32))
    w_e = np.ascontiguousarray(np.asarray(W_e, dtype=np.float32))
    w_n = np.ascontiguousarray(np.asarray(W_n, dtype=np.float32))
    be = np.ascontiguousarray(np.asarray(b_e, dtype=np.float32).reshape(HALF, 1))
    bn = np.ascontiguousarray(np.asarray(b_n, dtype=np.float32).reshape(HALF, 1))

    in_maps = []
    for c in range(N_CORES):
        sl = slice(c * NS, (c + 1) * NS)
        in_maps.append(
            {
                "e_vw": np.ascontiguousarray(e_vw[:, :, sl]),
                "h_w": np.ascontiguousarray(h_w[:, :, sl]),
                "W_e": w_e,
                "W_n": w_n,
                "b_e": be,
                "b_n": bn,
            }
        )
    res = run_bass_kernel_spmd(
        nc, in_maps, core_ids=list(range(N_CORES)), trace=trace, **kwargs
    )
    full = np.concatenate([res.results[c]["out"] for c in range(N_CORES)], axis=2)
    return full, res


def kernel(h_v=None, h_w=None, e_vw=None, W_e=None, b_e=None, W_n=None, b_n=None):
    full, _ = run(h_w, e_vw, W_e, b_e, W_n, b_n, trace=False)
    return full


# revision 10
# speedup vs baseline: 1.0570x; 1.0096x over previous
"""Trainium2 Bass kernel for nn_MessageFunction (GNN message passing).

reference:
    edge_out = einsum('ben,em->bmn', e_vw, W_e) + b_e   # [B, 128, N]
    node_out = einsum('bfn,fm->bmn', h_w,  W_n) + b_n   # [B, 128, N]
    out      = relu(concat([edge_out, node_out], axis=1))  # [B, 256, N]

h_v is an unused input (dead in the reference) — never transferred.

Sharding: data-parallel over the node axis (last dim) across 8 cores,
weights/biases replicated. Each core handles 6250 nodes:
  - stream [128, 1250] fp32 tiles of e_vw / h_w per batch,
  - matmul against resident 128x128 weights (fp32, K=128 contraction),
  - bias + ReLU fused: edge half on ScalarE (activation Relu w/ bias),
    node half on VectorE (tensor_scalar add+max) so the two engines
    run in parallel,
  - one combined [128, 2, 1250] DMA writes both halves of the output.
"""

import numpy as np

import concourse.bass as bass
import concourse.mybir as mybir
import concourse.tile as tile
from concourse import bacc
from concourse.bass_utils import run_bass_kernel_spmd

N_CORES = 8
B = 4
F = 128      # EDGE_F == NODE_F (contraction dim)
HALF = 128   # output channels per linear
N_NODES = 50000
NS = N_NODES // N_CORES       # 6250 nodes per core
T_DMA = 1250                  # free-dim tile per DMA (5 per batch slab)
N_T = NS // T_DMA             # 5

# Per-batch tile widths. First/last batch tapered: small tiles at the
# global pipeline start (stores ramp up sooner) and end (shorter drain).
_BODY = [T_DMA] * N_T                     # [1250]*5
_TILES = {
    0: [625, 625] + [T_DMA] * (N_T - 1),  # tapered start
    B - 1: [T_DMA] * (N_T - 1) + [625, 625],  # tapered end
}


def _mm_splits(width):
    # <=512 fp32 per matmul (one PSUM bank); near-uniform splits
    n = -(-width // 512)
    base, rem = divmod(width, n)
    return [base + (1 if i < rem else 0) for i in range(n)]

_FP32 = mybir.dt.float32

_compiled = None


def _build():
    nc = bacc.Bacc(
        "TRN2",
        target_bir_lowering=False,
        debug=False,
        num_devices=N_CORES,
    )
    e_vw = nc.dram_tensor("e_vw", (B, F, NS), _FP32, kind="ExternalInput").ap()
    h_w = nc.dram_tensor("h_w", (B, F, NS), _FP32, kind="ExternalInput").ap()
    W_e = nc.dram_tensor("W_e", (F, HALF), _FP32, kind="ExternalInput").ap()
    W_n = nc.dram_tensor("W_n", (F, HALF), _FP32, kind="ExternalInput").ap()
    b_e = nc.dram_tensor("b_e", (HALF, 1), _FP32, kind="ExternalInput").ap()
    b_n = nc.dram_tensor("b_n", (HALF, 1), _FP32, kind="ExternalInput").ap()
    out = nc.dram_tensor("out", (B, 2 * HALF, NS), _FP32, kind="ExternalOutput").ap()

    relu = mybir.ActivationFunctionType.Relu
    alu_add = mybir.AluOpType.add
    alu_max = mybir.AluOpType.max

    with tile.TileContext(nc) as tc:
        with (
            tc.tile_pool(name="consts", bufs=1) as cpool,
            tc.tile_pool(name="xin", bufs=10) as inpool,
            tc.tile_pool(name="xout", bufs=6) as outpool,
            tc.tile_pool(name="psum", bufs=8, space="PSUM") as pspool,
        ):
            w_e_sb = cpool.tile([F, HALF], _FP32, tag="w_e")
            w_n_sb = cpool.tile([F, HALF], _FP32, tag="w_n")
            b_e_sb = cpool.tile([HALF, 1], _FP32, tag="b_e")
            b_n_sb = cpool.tile([HALF, 1], _FP32, tag="b_n")
            # constants via SWDGE so the sync HWDGE ring starts with the
            # first big streaming load
            nc.gpsimd.dma_start(w_e_sb[:], W_e)
            nc.gpsimd.dma_start(w_n_sb[:], W_n)
            nc.gpsimd.dma_start(b_e_sb[:], b_e)
            nc.gpsimd.dma_start(b_n_sb[:], b_n)

            for bb in range(B):
                n0 = 0
                for width in _TILES.get(bb, _BODY):
                    sl = bass.ds(n0, width)
                    n0 += width
                    e_t = inpool.tile([F, T_DMA], _FP32, tag="e")
                    h_t = inpool.tile([F, T_DMA], _FP32, tag="h")
                    nc.sync.dma_start(e_t[:, :width], e_vw[bb, :, sl])
                    nc.sync.dma_start(h_t[:, :width], h_w[bb, :, sl])
                    o_e = outpool.tile([F, T_DMA], _FP32, tag="oe")
                    o_n = outpool.tile([F, T_DMA], _FP32, tag="on")
                    # all edge matmuls first, then all node matmuls: fewer
                    # weight-buffer alternations on PE
                    c0 = 0
                    for w in _mm_splits(width):
                        ps_e = pspool.tile([HALF, 512], _FP32, tag="ps")
                        nc.tensor.matmul(ps_e[:, :w], w_e_sb[:], e_t[:, c0 : c0 + w])
                        nc.scalar.activation(
                            o_e[:, c0 : c0 + w],
                            ps_e[:, :w],
                            relu,
                            bias=b_e_sb[:, 0:1],
                        )
                        c0 += w
                    # edge-half store from ACT's HWDGE ring: depends only on
                    # ACT's own output, so no cross-engine HOL
                    nc.scalar.dma_start(out[bb, 0:HALF, sl], o_e[:, :width])
                    c0 = 0
                    for w in _mm_splits(width):
                        ps_n = pspool.tile([HALF, 512], _FP32, tag="ps")
                        nc.tensor.matmul(ps_n[:, :w], w_n_sb[:], h_t[:, c0 : c0 + w])
                        nc.vector.tensor_scalar(
                            o_n[:, c0 : c0 + w],
                            ps_n[:, :w],
                            b_n_sb[:, 0:1],
                            0.0,
                            alu_add,
                            alu_max,
                        )
                        c0 += w
                    # node-half store on SWDGE (gpsimd is otherwise idle)
                    nc.gpsimd.dma_start(out[bb, HALF : 2 * HALF, sl], o_n[:, :width])

    nc.compile()
    return nc


def _get_nc():
    global _compiled
    if _compiled is None:
        _compiled = _build()
    return _compiled


def run(h_w, e_vw, W_e, b_e, W_n, b_n, trace=False, **kwargs):
    nc = _get_nc()
    h_w = np.ascontiguousarray(np.asarray(h_w, dtype=np.float32))
    e_vw = np.ascontiguousarray(np.asarray(e_vw, dtype=np.float32))
    w_e = np.ascontiguousarray(np.asarray(W_e, dtype=np.float32))
    w_n = np.ascontiguousarray(np.asarray(W_n, dtype=np.float32))
    be = np.ascontiguousarray(np.asarray(b_e, dtype=np.float32).reshape(HALF, 1))
    bn = np.ascontiguousarray(np.asarray(b_n, dtype=np.float32).reshape(HALF, 1))

    in_maps = []
    for c in range(N_CORES):
        sl = slice(c * NS, (c + 1) * NS)
        in_maps.append(
            {
                "e_vw": np.ascontiguousarray(e_vw[:, :, sl]),
                "h_w": np.ascontiguousarray(h_w[:, :, sl]),
                "W_e": w_e,
                "W_n": w_n,
                "b_e": be,
                "b_n": bn,
            }
        )
    res = run_bass_kernel_spmd(
        nc, in_maps, core_ids=list(range(N_CORES)), trace=trace, **kwargs
    )
    full = np.concatenate([res.results[c]["out"] for c in range(N_CORES)], axis=2)
    return full, res


def kernel(h_v=None, h_w=None, e_vw=None, W_e=None, b_e=None, W_n=None, b_n=None):
    full, _ = run(h_w, e_vw, W_e, b_e, W_n, b_n, trace=False)
    return full


# revision 11
# speedup vs baseline: 1.2502x; 1.1828x over previous
"""Trainium2 Bass kernel for nn_MessageFunction (GNN message passing).

reference:
    edge_out = einsum('ben,em->bmn', e_vw, W_e) + b_e   # [B, 128, N]
    node_out = einsum('bfn,fm->bmn', h_w,  W_n) + b_n   # [B, 128, N]
    out      = relu(concat([edge_out, node_out], axis=1))  # [B, 256, N]

h_v is an unused input (dead in the reference) — never transferred.

Sharding: data-parallel over the node axis (last dim) across 8 cores,
weights/biases replicated. Each core handles 6250 nodes:
  - stream [128, 1250] fp32 tiles of e_vw / h_w per batch,
  - matmul against resident 128x128 weights (fp32, K=128 contraction),
  - bias + ReLU fused: edge half on ScalarE (activation Relu w/ bias),
    node half on VectorE (tensor_scalar add+max) so the two engines
    run in parallel,
  - one combined [128, 2, 1250] DMA writes both halves of the output.
"""

import numpy as np

import concourse.bass as bass
import concourse.mybir as mybir
import concourse.tile as tile
from concourse import bacc
from concourse.bass_utils import run_bass_kernel_spmd

N_CORES = 8
B = 4
F = 128      # EDGE_F == NODE_F (contraction dim)
HALF = 128   # output channels per linear
N_NODES = 50000
NS = N_NODES // N_CORES       # 6250 nodes per core
T_DMA = 1250                  # free-dim tile per DMA (5 per batch slab)
N_T = NS // T_DMA             # 5

# Per-batch tile widths. First/last batch tapered: small tiles at the
# global pipeline start (stores ramp up sooner) and end (shorter drain).
_BODY = [T_DMA] * N_T                     # [1250]*5
_TILES = {
    0: [625, 625] + [T_DMA] * (N_T - 1),  # tapered start
    B - 1: [T_DMA] * (N_T - 1) + [625, 625],  # tapered end
}


def _mm_splits(width):
    # <=512 fp32 per matmul (one PSUM bank); near-uniform splits
    n = -(-width // 512)
    base, rem = divmod(width, n)
    return [base + (1 if i < rem else 0) for i in range(n)]

_FP32 = mybir.dt.float32

_compiled = None


def _build():
    nc = bacc.Bacc(
        "TRN2",
        target_bir_lowering=False,
        debug=False,
        num_devices=N_CORES,
    )
    e_vw = nc.dram_tensor("e_vw", (B, F, NS), _FP32, kind="ExternalInput").ap()
    h_w = nc.dram_tensor("h_w", (B, F, NS), _FP32, kind="ExternalInput").ap()
    W_e = nc.dram_tensor("W_e", (F, HALF), _FP32, kind="ExternalInput").ap()
    W_n = nc.dram_tensor("W_n", (F, HALF), _FP32, kind="ExternalInput").ap()
    b_e = nc.dram_tensor("b_e", (HALF, 1), _FP32, kind="ExternalInput").ap()
    b_n = nc.dram_tensor("b_n", (HALF, 1), _FP32, kind="ExternalInput").ap()
    out = nc.dram_tensor("out", (B, 2 * HALF, NS), _FP32, kind="ExternalOutput").ap()

    relu = mybir.ActivationFunctionType.Relu
    alu_add = mybir.AluOpType.add
    alu_max = mybir.AluOpType.max

    with tile.TileContext(nc) as tc:
        with (
            tc.tile_pool(name="consts", bufs=1) as cpool,
            tc.tile_pool(name="xin", bufs=8) as inpool,
            tc.tile_pool(name="xout", bufs=4) as outpool,
            tc.tile_pool(name="psum", bufs=8, space="PSUM") as pspool,
        ):
            w_e_sb = cpool.tile([F, HALF], _FP32, tag="w_e")
            w_n_sb = cpool.tile([F, HALF], _FP32, tag="w_n")
            b_e_sb = cpool.tile([HALF, 1], _FP32, tag="b_e")
            b_n_sb = cpool.tile([HALF, 1], _FP32, tag="b_n")
            # constants via SWDGE so the sync HWDGE ring starts with the
            # first big streaming load
            nc.gpsimd.dma_start(w_e_sb[:], W_e)
            nc.gpsimd.dma_start(w_n_sb[:], W_n)
            nc.gpsimd.dma_start(b_e_sb[:], b_e)
            nc.gpsimd.dma_start(b_n_sb[:], b_n)

            for bb in range(B):
                n0 = 0
                for width in _TILES.get(bb, _BODY):
                    sl = bass.ds(n0, width)
                    n0 += width
                    e_t = inpool.tile([F, T_DMA], _FP32, tag="e")
                    h_t = inpool.tile([F, T_DMA], _FP32, tag="h")
                    nc.sync.dma_start(e_t[:, :width], e_vw[bb, :, sl])
                    nc.sync.dma_start(h_t[:, :width], h_w[bb, :, sl])
                    o_e = outpool.tile([F, T_DMA], _FP32, tag="oe")
                    o_n = outpool.tile([F, T_DMA], _FP32, tag="on")
                    # all edge matmuls first, then all node matmuls: fewer
                    # weight-buffer alternations on PE
                    c0 = 0
                    for w in _mm_splits(width):
                        ps_e = pspool.tile([HALF, 512], _FP32, tag="ps")
                        nc.tensor.matmul(ps_e[:, :w], w_e_sb[:], e_t[:, c0 : c0 + w])
                        nc.scalar.activation(
                            o_e[:, c0 : c0 + w],
                            ps_e[:, :w],
                            relu,
                            bias=b_e_sb[:, 0:1],
                        )
                        c0 += w
                    # edge-half store from ACT's HWDGE ring: depends only on
                    # ACT's own output, so no cross-engine HOL
                    nc.scalar.dma_start(out[bb, 0:HALF, sl], o_e[:, :width])
                    c0 = 0
                    for w in _mm_splits(width):
                        ps_n = pspool.tile([HALF, 512], _FP32, tag="ps")
                        nc.tensor.matmul(ps_n[:, :w], w_n_sb[:], h_t[:, c0 : c0 + w])
                        nc.vector.tensor_scalar(
                            o_n[:, c0 : c0 + w],
                            ps_n[:, :w],
                            b_n_sb[:, 0:1],
                            0.0,
                            alu_add,
                            alu_max,
                        )
                        c0 += w
                    # node-half store on SWDGE (gpsimd is otherwise idle)
                    nc.gpsimd.dma_start(out[bb, HALF : 2 * HALF, sl], o_n[:, :width])

    nc.compile()
    return nc


def _get_nc():
    global _compiled
    if _compiled is None:
        _compiled = _build()
    return _compiled


def run(h_w, e_vw, W_e, b_e, W_n, b_n, trace=False, **kwargs):
    nc = _get_nc()
    h_w = np.ascontiguousarray(np.asarray(h_w, dtype=np.float32))
    e_vw = np.ascontiguousarray(np.asarray(e_vw, dtype=np.float32))
    w_e = np.ascontiguousarray(np.asarray(W_e, dtype=np.float32))
    w_n = np.ascontiguousarray(np.asarray(W_n, dtype=np.float32))
    be = np.ascontiguousarray(np.asarray(b_e, dtype=np.float32).reshape(HALF, 1))
    bn = np.ascontiguousarray(np.asarray(b_n, dtype=np.float32).reshape(HALF, 1))

    in_maps = []
    for c in range(N_CORES):
        sl = slice(c * NS, (c + 1) * NS)
        in_maps.append(
            {
                "e_vw": np.ascontiguousarray(e_vw[:, :, sl]),
                "h_w": np.ascontiguousarray(h_w[:, :, sl]),
                "W_e": w_e,
                "W_n": w_n,
                "b_e": be,
                "b_n": bn,
            }
        )
    res = run_bass_kernel_spmd(
        nc, in_maps, core_ids=list(range(N_CORES)), trace=trace, **kwargs
    )
    full = np.concatenate([res.results[c]["out"] for c in range(N_CORES)], axis=2)
    return full, res


def kernel(h_v=None, h_w=None, e_vw=None, W_e=None, b_e=None, W_n=None, b_n=None):
    full, _ = run(h_w, e_vw, W_e, b_e, W_n, b_n, trace=False)
    return full
